# revision 13
# baseline (speedup 1.0000x reference)
"""Multi-head GQA attention (B=2, S=2048, H=4096, 32 q-heads / 8 kv-heads,
HD=128, rotary, causal) on 8 TRN2 NeuronCores.

Sharding: tensor-parallel over heads, 8-way — core c owns q-heads
[4c, 4c+4) and kv-head c; wq/wk/wv column-sharded, wo row-sharded.  Each
core computes a partial wo product over its head slice for both batches;
the host sums the 8 partials (the TP reduction) and transposes back.

All on-device dataflow is in transposed layout [feature, seq] so no
device-side transposes are needed; the host pre-transposes x and the
weight shards during sharding.  Rotary pairs are laid out so the (x0,x1)
pair swap is a 32-partition-quadrant stream_shuffle on the Vector engine.

Engine assignment keeps the in-order ACT/DVE queues off the PE's
critical path: GpSimd issues all steady-state DMAs (25ns vs ~600ns on
SP/ACT) and takes half the wo-output PSUM->SBUF copies plus the V
transpose copies; ACT only does rotary PSUM copies and exp; DVE does
rotary arithmetic (bf16, 2x mode), masking, normalization, and the other
half of the wo copies.  Softmax denominators come from an all-ones
[128,128] stationary matmul accumulating a broadcast [128,512] sum
directly (no [1,512] row, no separate broadcast matmul), emitted as a
pure-PE pass after the PV accumulation.  Fully-masked regions of
diagonal blocks are skipped in the scores/exp/PV/sum pipeline.
"""
import sys

if "/opt/trn_rl_repo" not in sys.path:
    sys.path.insert(0, "/opt/trn_rl_repo")

import numpy as np
import ml_dtypes

from concourse import bacc, tile, mybir
from concourse.bass_utils import run_bass_kernel_spmd

F32 = mybir.dt.float32
BF16 = mybir.dt.bfloat16
EXP = mybir.ActivationFunctionType.Exp
COPY = mybir.ActivationFunctionType.Copy
BF16NP = ml_dtypes.bfloat16

B, S, H = 2, 2048, 4096
NH, NKV, HD = 32, 8, 128
NCORES = 8
QH = NH // NCORES          # 4 q-heads per core
RQ = QH * HD               # 512 q rows per core
SB = 512                   # projection seq block
NSB = S // SB              # 4
IB = 512                   # attention i block
NIB = S // IB              # 4
HC = H // 128              # 32 contraction chunks
NJT = S // 128             # 16 j tiles

# stream_shuffle mask: swap 16-partition halves within each 32-partition quadrant
SHUF = list(range(16, 32)) + list(range(16))

LAST_EXEC_NS = None
_CACHED = None


def _build(dbg=False):
    nc = bacc.Bacc("TRN2", target_bir_lowering=False, debug=False,
                   num_devices=NCORES)

    xt_e = nc.dram_tensor("xt", [B, H, S], BF16, kind="ExternalInput")
    cc_e = nc.dram_tensor("cc", [B, 128, S], BF16, kind="ExternalInput")
    ss_e = nc.dram_tensor("ss2", [B, 128, S], BF16, kind="ExternalInput")
    wq_e = nc.dram_tensor("wqt", [H, RQ], BF16, kind="ExternalInput")
    wk_e = nc.dram_tensor("wkt", [H, HD], BF16, kind="ExternalInput")
    wv_e = nc.dram_tensor("wvt", [H, HD], BF16, kind="ExternalInput")
    wo_e = nc.dram_tensor("wot", [RQ, H], BF16, kind="ExternalInput")
    tm_e = nc.dram_tensor("trimask", [128, 128], BF16, kind="ExternalInput")
    id_e = nc.dram_tensor("ident", [128, 128], BF16, kind="ExternalInput")
    out_e = nc.dram_tensor("out", [B, H, S], BF16, kind="ExternalOutput")

    with tile.TileContext(nc) as tc:
        with (nc.allow_low_precision(reason="bf16 compute by design"),
              tc.tile_pool(name="wpool", bufs=1) as wp,
              tc.tile_pool(name="state", bufs=1) as st,
              tc.tile_pool(name="att", bufs=2) as ap_,
              tc.tile_pool(name="xin", bufs=3) as xp,
              tc.tile_pool(name="probs", bufs=1) as pp,
              tc.tile_pool(name="rot", bufs=2) as rp,
              tc.tile_pool(name="stage", bufs=2) as sg,
              tc.tile_pool(name="ps", bufs=6, space="PSUM") as ps):

            # ---- resident weights ----
            wq_s = wp.tile([128, HC, RQ], BF16)
            wk_s = wp.tile([128, HC, HD], BF16)
            wv_s = wp.tile([128, HC, HD], BF16)
            wo_s = wp.tile([128, QH, H], BF16)
            tm_s = wp.tile([128, 128], BF16)
            ones_s = wp.tile([128, 128], BF16)
            id_s = wp.tile([128, 128], BF16)
            nc.vector.memset(ones_s[:], 1.0)

            # weight chunks on the Sync queue (startup only); x/cc/ss/out
            # on the GpSimd queue whose DMA issue is ~25ns
            _wchunk_loaded = [False] * (HC // 4)
            _tail_loaded = [False]

            def load_wchunk(hg):
                if _wchunk_loaded[hg]:
                    return
                _wchunk_loaded[hg] = True
                if hg == 0:
                    # single-chunk contiguous DMAs so the first matmul's
                    # dependencies land with minimal issue+transfer latency
                    nc.sync.dma_start(out=wq_s[:, 0], in_=wq_e.ap()[0:128])
                    nc.sync.dma_start(out=wk_s[:, 0], in_=wk_e.ap()[0:128])
                    nc.sync.dma_start(out=wv_s[:, 0], in_=wv_e.ap()[0:128])
                    return
                r0, r1 = hg * 512, (hg + 1) * 512
                nc.sync.dma_start(
                    out=wq_s[:, 4 * hg:4 * hg + 4],
                    in_=wq_e.ap()[r0:r1].rearrange("(c p) r -> p c r", c=4))
                nc.sync.dma_start(
                    out=wk_s[:, 4 * hg:4 * hg + 4],
                    in_=wk_e.ap()[r0:r1].rearrange("(c p) r -> p c r", c=4))
                nc.sync.dma_start(
                    out=wv_s[:, 4 * hg:4 * hg + 4],
                    in_=wv_e.ap()[r0:r1].rearrange("(c p) r -> p c r", c=4))

            def load_wchunk0_rest():
                # chunks 1-3, issued after the first x tile's DMA
                nc.sync.dma_start(
                    out=wq_s[:, 1:4],
                    in_=wq_e.ap()[128:512].rearrange("(c p) r -> p c r", c=3))
                nc.sync.dma_start(
                    out=wk_s[:, 1:4],
                    in_=wk_e.ap()[128:512].rearrange("(c p) r -> p c r", c=3))
                nc.sync.dma_start(
                    out=wv_s[:, 1:4],
                    in_=wv_e.ap()[128:512].rearrange("(c p) r -> p c r", c=3))

            def load_tail_weights():
                if _tail_loaded[0]:
                    return
                _tail_loaded[0] = True
                nc.sync.dma_start(out=id_s[:], in_=id_e.ap())
                nc.sync.dma_start(out=tm_s[:], in_=tm_e.ap())
                for rc in range(QH):
                    nc.sync.dma_start(out=wo_s[:, rc],
                                      in_=wo_e.ap()[rc * 128:(rc + 1) * 128])

            _rot_n = [0]

            def rotary(src_ps, s0, ccb, ssb, dst_ap):
                # dst = src*CC + shuffle(src)*SS2, bf16 (DVE 2x mode)
                r = _rot_n[0] = (_rot_n[0] + 1) % 2
                qs = rp.tile([128, SB], BF16, tag=f"qs{r}", bufs=1)
                nc.scalar.activation(qs[:], src_ps[:], COPY)
                qw = rp.tile([128, SB], BF16, tag=f"qw{r}", bufs=1)
                nc.vector.stream_shuffle(qw[:], qs[:], SHUF)
                nc.vector.tensor_mul(qs[:], qs[:], ccb[:, s0:s0 + SB])
                nc.vector.tensor_mul(qw[:], qw[:], ssb[:, s0:s0 + SB])
                nc.vector.tensor_add(dst_ap, qs[:], qw[:])

            pending_wo = []

            def emit_wo(wb, wib, wat, ho_lo, ho_hi):
                wi0 = wib * IB
                for ho in range(ho_lo, min(ho_hi, H // 128)):
                    acc = ps.tile([128, IB], F32, tag="ps", name="acc")
                    for rc in range(QH):
                        nc.tensor.matmul(acc[:],
                                         wo_s[:, rc, ho * 128:(ho + 1) * 128],
                                         wat[:, rc],
                                         start=(rc == 0), stop=(rc == QH - 1))
                    og = sg.tile([128, IB], BF16, tag=f"og{ho % 4}", name="og",
                                 bufs=1)
                    nc.vector.tensor_copy(og[:], acc[:])
                    nc.sync.dma_start(
                        out=out_e.ap()[wb, ho * 128:(ho + 1) * 128, wi0:wi0 + IB],
                        in_=og[:])

            for b in range(B):
                qT = st.tile([128, QH, S], BF16, tag="qT")
                kT = st.tile([128, S], BF16, tag="kT")
                vn = st.tile([128, NJT, HD], BF16, tag="vn")
                ccb = st.tile([128, S], BF16, tag="cc")
                ssb = st.tile([128, S], BF16, tag="ss")
                nc.scalar.dma_start(out=ccb[:], in_=cc_e.ap()[b])
                nc.scalar.dma_start(out=ssb[:], in_=ss_e.ap()[b])

                # interleaved: projection block sb feeds attention block
                # ib == sb immediately (causal: ib needs only j <= (sb+1)*SB)
                for sb in range(NSB):
                    s0 = sb * SB
                    # ---- projection for seq block sb ----
                    qa = [ps.tile([128, SB], F32, tag="ps", name=f"qa{rc}")
                          for rc in range(QH)]
                    ka = ps.tile([128, SB], F32, tag="ps")
                    vta = ps.tile([128, SB], F32, tag="ps")
                    for hg in range(HC // 4):
                        first = not _wchunk_loaded[hg] and hg == 0
                        load_wchunk(hg)
                        xt_g = xp.tile([128, 4, SB], BF16, tag=f"xt{hg % 4}",
                                       bufs=1)
                        eng = nc.sync if hg % 2 == 0 else nc.scalar
                        eng.dma_start(
                            out=xt_g[:],
                            in_=xt_e.ap()[b, hg * 512:(hg + 1) * 512, s0:s0 + SB]
                            .rearrange("(c p) s -> p c s", c=4))
                        if first:
                            load_wchunk0_rest()
                        for c in range(4):
                            hc = hg * 4 + c
                            st_, sp_ = (hc == 0), (hc == HC - 1)
                            for rc in range(QH):
                                nc.tensor.matmul(qa[rc][:],
                                                 wq_s[:, hc, rc * 128:(rc + 1) * 128],
                                                 xt_g[:, c], start=st_, stop=sp_)
                            nc.tensor.matmul(ka[:], wk_s[:, hc], xt_g[:, c],
                                             start=st_, stop=sp_)
                            nc.tensor.matmul(vta[:], wv_s[:, hc], xt_g[:, c],
                                             start=st_, stop=sp_)
                    load_tail_weights()
                    # vt_s copy first on ACT so the V transposes (below,
                    # after the wo slice) never wait on the rotary copies
                    vt_s = rp.tile([128, SB], BF16, tag="vts")
                    nc.scalar.activation(vt_s[:], vta[:], COPY)
                    rotary(ka, s0, ccb, ssb, kT[:, s0:s0 + SB])
                    for rc in range(QH):
                        rotary(qa[rc], s0, ccb, ssb, qT[:, rc, s0:s0 + SB])

                    # previous block's first wo slice: independent PE work
                    # that covers the rotary-chain latency on ACT/DVE; the
                    # rest is interleaved into the attention head loop below
                    prev = pending_wo.pop() if pending_wo else None
                    if prev is not None:
                        emit_wo(*prev, 0, 8)

                    # vT [d, s] -> natural v j-tiles via PE transpose (bf16)
                    for t in range(SB // 128):
                        tp = ps.tile([128, 128], BF16, tag="ps", name="tp",
                                     padded_shape=[128, IB * 2])
                        nc.tensor.transpose(tp[:], vt_s[:, t * 128:(t + 1) * 128],
                                            id_s[:])
                        nc.scalar.activation(vn[:, (SB // 128) * sb + t], tp[:],
                                             COPY)

                    # ---- attention for i block ib == sb ----
                    ib = sb
                    i0 = ib * IB
                    njt = (ib + 1) * (IB // 128)
                    at = ap_.tile([128, QH, IB], BF16, tag="at")
                    for h in range(QH):
                        pv = ps.tile([128, IB], F32, tag="ps")
                        sumb = ps.tile([128, IB], F32, tag="ps")
                        pend = []
                        pbs = []
                        sums_ops = []

                        def flush(stop):
                            pjt, pc0, ppb = pend.pop(0)
                            nc.tensor.matmul(pv[:, pc0:], vn[:, pjt],
                                             ppb[:, pc0:],
                                             start=(pjt == 0), stop=stop)
                        for jt in range(njt):
                            kd = jt - ib * 4
                            c0 = kd * 128 if kd > 0 else 0
                            sc = ps.tile([128, IB], F32, tag=f"sc{jt % 2}",
                                         name="sc", bufs=1)
                            nc.tensor.matmul(sc[:, c0:],
                                             kT[:, jt * 128:(jt + 1) * 128],
                                             qT[:, h, i0 + c0:i0 + IB],
                                             start=True, stop=True)
                            pb = pp.tile([128, IB], BF16, tag=f"pb{jt}",
                                         bufs=1)
                            nc.scalar.activation(pb[:, c0:], sc[:, c0:], EXP)
                            if kd >= 0:
                                nc.vector.tensor_mul(pb[:, c0:c0 + 128],
                                                     pb[:, c0:c0 + 128],
                                                     tm_s[:])
                            pend.append((jt, c0, pb))
                            pbs.append(pb)
                            if kd < 0 and jt % 2 == 1:
                                # pre-add full-width pairs on DVE so the sums
                                # pass below needs half the matmuls
                                pa = pp.tile([128, IB], BF16,
                                             tag=f"pa{(jt // 2) % 8}", bufs=1)
                                nc.vector.tensor_add(pa[:], pbs[jt - 1][:],
                                                     pb[:])
                                sums_ops.append((0, pa))
                            elif kd >= 0:
                                sums_ops.append((c0, pb))
                            if len(pend) > 2:
                                flush(False)
                        while pend:
                            flush(stop=(len(pend) == 1))
                        # softmax denominators: all-ones stationary sums pb
                        # over j, broadcast to all 128 partitions.  Pure PE
                        # work, scheduled after PV so it fills exp bubbles.
                        for k, (c0, src) in enumerate(sums_ops):
                            nc.tensor.matmul(sumb[:, c0:], ones_s[:],
                                             src[:, c0:],
                                             start=(k == 0),
                                             stop=(k == len(sums_ops) - 1))
                        rb = sg.tile([128, IB], F32, tag=f"rb{h % 2}", bufs=1)
                        nc.vector.reciprocal_approx_fast(rb[:], sumb[:])
                        nc.vector.tensor_mul(at[:, h], pv[:], rb[:])
                        # next slice of the previous block's wo: PE slack
                        # work covering this head's recip/normalize chain
                        if prev is not None:
                            emit_wo(*prev, 8 * (h + 1), 8 * (h + 2))
                    pending_wo.append((b, ib, at))

            emit_wo(*pending_wo.pop(), 0, H // 128)

    nc.compile()
    return nc


def _prep(x, freqs_cos, freqs_sin, wq, wk, wv, wo):
    """Shard + pre-transpose inputs for the 8 cores."""
    # rotary pair permutation: within each 32-partition quadrant, x0 of
    # pairs [16q,16q+16) sits in local slots 0..15 and x1 in 16..31.
    perm = np.zeros(HD, dtype=np.int64)
    pair = np.zeros(128, dtype=np.int64)
    sign = np.zeros(128, dtype=np.float32)
    for q in range(4):
        for t in range(16):
            perm[32 * q + t] = 2 * (16 * q + t)
            perm[32 * q + 16 + t] = 2 * (16 * q + t) + 1
            pair[32 * q + t] = 16 * q + t
            pair[32 * q + 16 + t] = 16 * q + t
            sign[32 * q + t] = -1.0
            sign[32 * q + 16 + t] = 1.0

    xt = np.ascontiguousarray(x.transpose(0, 2, 1)).astype(BF16NP)  # [B,H,S]
    cc = np.ascontiguousarray(freqs_cos[:, :, pair].transpose(0, 2, 1)).astype(BF16NP)
    ss2 = np.ascontiguousarray((freqs_sin[:, :, pair] * sign).transpose(0, 2, 1)).astype(BF16NP)

    # causal mask for the 128-wide boundary sub-tile of a diagonal j tile
    jj = np.arange(128)[:, None]
    ii = np.arange(128)[None, :]
    trimask = (jj <= ii).astype(BF16NP)

    scale = np.float32(1.0 / np.sqrt(HD))
    in_maps = []
    for c in range(NCORES):
        wq_c = (wq[c * RQ:(c + 1) * RQ] * scale).reshape(QH, HD, H)[:, perm, :]
        wqt = np.ascontiguousarray(wq_c.reshape(RQ, H).T).astype(BF16NP)
        wk_c = wk[c * HD:(c + 1) * HD][perm, :]
        wkt = np.ascontiguousarray(wk_c.T).astype(BF16NP)
        wvt = np.ascontiguousarray(wv[c * HD:(c + 1) * HD].T).astype(BF16NP)
        wot = np.ascontiguousarray(wo[:, c * RQ:(c + 1) * RQ].T).astype(BF16NP)
        in_maps.append({
            "xt": xt, "cc": cc, "ss2": ss2,
            "wqt": wqt, "wkt": wkt, "wvt": wvt, "wot": wot,
            "trimask": trimask, "ident": np.eye(128, dtype=BF16NP),
        })
    return in_maps


def kernel(x, freqs_cos, freqs_sin, wq, wk, wv, wo):
    global _CACHED, LAST_EXEC_NS
    x = np.asarray(x, dtype=np.float32)
    freqs_cos = np.asarray(freqs_cos, dtype=np.float32)
    freqs_sin = np.asarray(freqs_sin, dtype=np.float32)
    wq = np.asarray(wq, dtype=np.float32)
    wk = np.asarray(wk, dtype=np.float32)
    wv = np.asarray(wv, dtype=np.float32)
    wo = np.asarray(wo, dtype=np.float32)

    if _CACHED is None:
        _CACHED = _build()
    nc = _CACHED

    in_maps = _prep(x, freqs_cos, freqs_sin, wq, wk, wv, wo)
    res = run_bass_kernel_spmd(nc, in_maps, core_ids=list(range(NCORES)))
    LAST_EXEC_NS = res.exec_time_ns

    # unshard: sum the 8 partial wo products, then [B,H,S] -> [B,S,H]
    acc = res.results[0]["out"].astype(np.float64)
    for c in range(1, NCORES):
        acc += res.results[c]["out"]
    return np.ascontiguousarray(acc.transpose(0, 2, 1)).astype(np.float32)


# revision 15
# speedup vs baseline: 1.1707x; 1.1707x over previous
"""Multi-head GQA attention (B=2, S=2048, H=4096, 32 q-heads / 8 kv-heads,
HD=128, rotary, causal) on 8 TRN2 NeuronCores.

Sharding: tensor-parallel over heads, 8-way — core c owns q-heads
[4c, 4c+4) and kv-head c; wq/wk/wv column-sharded, wo row-sharded.  Each
core computes a partial wo product over its head slice for both batches;
the host sums the 8 partials (the TP reduction) and transposes back.

All on-device dataflow is in transposed layout [feature, seq] so no
device-side transposes are needed; the host pre-transposes x and the
weight shards during sharding.  Rotary pairs are laid out so the (x0,x1)
pair swap is a 32-partition-quadrant stream_shuffle on the Vector engine.

Engine assignment keeps the in-order ACT/DVE queues off the PE's
critical path: GpSimd issues all steady-state DMAs (25ns vs ~600ns on
SP/ACT) and takes half the wo-output PSUM->SBUF copies plus the V
transpose copies; ACT only does rotary PSUM copies and exp; DVE does
rotary arithmetic (bf16, 2x mode), masking, normalization, and the other
half of the wo copies.  Softmax denominators come from an all-ones
[128,128] stationary matmul accumulating a broadcast [128,512] sum
directly (no [1,512] row, no separate broadcast matmul), emitted as a
pure-PE pass after the PV accumulation.  Fully-masked regions of
diagonal blocks are skipped in the scores/exp/PV/sum pipeline.
"""
import sys

if "/opt/trn_rl_repo" not in sys.path:
    sys.path.insert(0, "/opt/trn_rl_repo")

import numpy as np
import ml_dtypes

from concourse import bacc, tile, mybir
from concourse.bass_utils import run_bass_kernel_spmd

F32 = mybir.dt.float32
BF16 = mybir.dt.bfloat16
EXP = mybir.ActivationFunctionType.Exp
COPY = mybir.ActivationFunctionType.Copy
BF16NP = ml_dtypes.bfloat16

B, S, H = 2, 2048, 4096
NH, NKV, HD = 32, 8, 128
NCORES = 8
QH = NH // NCORES          # 4 q-heads per core
RQ = QH * HD               # 512 q rows per core
SB = 512                   # projection seq block
NSB = S // SB              # 4
IB = 512                   # attention i block
NIB = S // IB              # 4
HC = H // 128              # 32 contraction chunks
NJT = S // 128             # 16 j tiles

# stream_shuffle mask: swap 16-partition halves within each 32-partition quadrant
SHUF = list(range(16, 32)) + list(range(16))

LAST_EXEC_NS = None
_CACHED = None


def _build(dbg=False):
    nc = bacc.Bacc("TRN2", target_bir_lowering=False, debug=False,
                   num_devices=NCORES)

    xt_e = nc.dram_tensor("xt", [B, H, S], BF16, kind="ExternalInput")
    cc_e = nc.dram_tensor("cc", [B, 128, S], BF16, kind="ExternalInput")
    ss_e = nc.dram_tensor("ss2", [B, 128, S], BF16, kind="ExternalInput")
    wq_e = nc.dram_tensor("wqt", [H, RQ], BF16, kind="ExternalInput")
    wk_e = nc.dram_tensor("wkt", [H, HD], BF16, kind="ExternalInput")
    wv_e = nc.dram_tensor("wvt", [H, HD], BF16, kind="ExternalInput")
    wo_e = nc.dram_tensor("wot", [RQ, H], BF16, kind="ExternalInput")
    tm_e = nc.dram_tensor("trimask", [128, 128], BF16, kind="ExternalInput")
    id_e = nc.dram_tensor("ident", [128, 128], BF16, kind="ExternalInput")
    out_e = nc.dram_tensor("out", [B, H, S], BF16, kind="ExternalOutput")

    with tile.TileContext(nc) as tc:
        with (nc.allow_low_precision(reason="bf16 compute by design"),
              tc.tile_pool(name="wpool", bufs=1) as wp,
              tc.tile_pool(name="state", bufs=1) as st,
              tc.tile_pool(name="att", bufs=2) as ap_,
              tc.tile_pool(name="xin", bufs=3) as xp,
              tc.tile_pool(name="probs", bufs=1) as pp,
              tc.tile_pool(name="rot", bufs=2) as rp,
              tc.tile_pool(name="stage", bufs=2) as sg,
              tc.tile_pool(name="ps", bufs=6, space="PSUM") as ps):

            # ---- resident weights ----
            wq_s = wp.tile([128, HC, RQ], BF16)
            wk_s = wp.tile([128, HC, HD], BF16)
            wv_s = wp.tile([128, HC, HD], BF16)
            wo_s = wp.tile([128, QH, H], BF16)
            tm_s = wp.tile([128, 128], BF16)
            ones_s = wp.tile([128, 128], BF16)
            id_s = wp.tile([128, 128], BF16)
            nc.vector.memset(ones_s[:], 1.0)

            # weight chunks on the Sync queue (startup only); x/cc/ss/out
            # on the GpSimd queue whose DMA issue is ~25ns
            _wchunk_loaded = [False] * (HC // 4)
            _tail_loaded = [False]

            def load_wchunk(hg):
                if _wchunk_loaded[hg]:
                    return
                _wchunk_loaded[hg] = True
                if hg == 0:
                    # single-chunk contiguous DMAs so the first matmul's
                    # dependencies land with minimal issue+transfer latency
                    nc.sync.dma_start(out=wq_s[:, 0], in_=wq_e.ap()[0:128])
                    nc.sync.dma_start(out=wk_s[:, 0], in_=wk_e.ap()[0:128])
                    nc.sync.dma_start(out=wv_s[:, 0], in_=wv_e.ap()[0:128])
                    return
                r0, r1 = hg * 512, (hg + 1) * 512
                nc.sync.dma_start(
                    out=wq_s[:, 4 * hg:4 * hg + 4],
                    in_=wq_e.ap()[r0:r1].rearrange("(c p) r -> p c r", c=4))
                nc.sync.dma_start(
                    out=wk_s[:, 4 * hg:4 * hg + 4],
                    in_=wk_e.ap()[r0:r1].rearrange("(c p) r -> p c r", c=4))
                nc.sync.dma_start(
                    out=wv_s[:, 4 * hg:4 * hg + 4],
                    in_=wv_e.ap()[r0:r1].rearrange("(c p) r -> p c r", c=4))

            def load_wchunk0_rest():
                # chunks 1-3, issued after the first x tile's DMA
                nc.sync.dma_start(
                    out=wq_s[:, 1:4],
                    in_=wq_e.ap()[128:512].rearrange("(c p) r -> p c r", c=3))
                nc.sync.dma_start(
                    out=wk_s[:, 1:4],
                    in_=wk_e.ap()[128:512].rearrange("(c p) r -> p c r", c=3))
                nc.sync.dma_start(
                    out=wv_s[:, 1:4],
                    in_=wv_e.ap()[128:512].rearrange("(c p) r -> p c r", c=3))

            def load_tail_weights():
                if _tail_loaded[0]:
                    return
                _tail_loaded[0] = True
                nc.sync.dma_start(out=id_s[:], in_=id_e.ap())
                nc.sync.dma_start(out=tm_s[:], in_=tm_e.ap())
                for rc in range(QH):
                    nc.sync.dma_start(out=wo_s[:, rc],
                                      in_=wo_e.ap()[rc * 128:(rc + 1) * 128])

            _rot_n = [0]

            def rotary(src_ps, s0, ccb, ssb, dst_ap):
                # dst = src*CC + shuffle(src)*SS2, bf16 (DVE 2x mode)
                r = _rot_n[0] = (_rot_n[0] + 1) % 2
                qs = rp.tile([128, SB], BF16, tag=f"qs{r}", bufs=1)
                nc.scalar.activation(qs[:], src_ps[:], COPY)
                qw = rp.tile([128, SB], BF16, tag=f"qw{r}", bufs=1)
                nc.vector.stream_shuffle(qw[:], qs[:], SHUF)
                nc.vector.tensor_mul(qs[:], qs[:], ccb[:, s0:s0 + SB])
                nc.vector.tensor_mul(qw[:], qw[:], ssb[:, s0:s0 + SB])
                nc.vector.tensor_add(dst_ap, qs[:], qw[:])

            pending_wo = []

            def emit_wo(wb, wib, wat, ho_lo, ho_hi):
                wi0 = wib * IB
                for ho in range(ho_lo, min(ho_hi, H // 128)):
                    acc = ps.tile([128, IB], F32, tag="ps", name="acc")
                    for rc in range(QH):
                        nc.tensor.matmul(acc[:],
                                         wo_s[:, rc, ho * 128:(ho + 1) * 128],
                                         wat[:, rc],
                                         start=(rc == 0), stop=(rc == QH - 1))
                    og = sg.tile([128, IB], BF16, tag=f"og{ho % 4}", name="og",
                                 bufs=1)
                    if ho % 2 == 0:
                        nc.scalar.activation(og[:], acc[:], COPY)
                    else:
                        nc.vector.tensor_copy(og[:], acc[:])
                    oeng = nc.scalar if ho % 2 == 0 else nc.sync
                    oeng.dma_start(
                        out=out_e.ap()[wb, ho * 128:(ho + 1) * 128, wi0:wi0 + IB],
                        in_=og[:])

            for b in range(B):
                qT = st.tile([128, QH, S], BF16, tag="qT")
                kT = st.tile([128, S], BF16, tag="kT")
                vn = st.tile([128, NJT, HD], BF16, tag="vn")
                ccb = st.tile([128, S], BF16, tag="cc")
                ssb = st.tile([128, S], BF16, tag="ss")
                nc.scalar.dma_start(out=ccb[:], in_=cc_e.ap()[b])
                nc.scalar.dma_start(out=ssb[:], in_=ss_e.ap()[b])

                # interleaved: projection block sb feeds attention block
                # ib == sb immediately (causal: ib needs only j <= (sb+1)*SB)
                for sb in range(NSB):
                    s0 = sb * SB
                    # ---- projection for seq block sb ----
                    qa = [ps.tile([128, SB], F32, tag="ps", name=f"qa{rc}")
                          for rc in range(QH)]
                    ka = ps.tile([128, SB], F32, tag="ps")
                    vta = ps.tile([128, SB], F32, tag="ps")
                    for hg in range(HC // 4):
                        first = not _wchunk_loaded[hg] and hg == 0
                        load_wchunk(hg)
                        xt_g = xp.tile([128, 4, SB], BF16, tag=f"xt{hg % 4}",
                                       bufs=1)
                        eng = nc.sync if hg % 2 == 0 else nc.scalar
                        eng.dma_start(
                            out=xt_g[:],
                            in_=xt_e.ap()[b, hg * 512:(hg + 1) * 512, s0:s0 + SB]
                            .rearrange("(c p) s -> p c s", c=4))
                        if first:
                            load_wchunk0_rest()
                        for c in range(4):
                            hc = hg * 4 + c
                            st_, sp_ = (hc == 0), (hc == HC - 1)
                            for rc in range(QH):
                                nc.tensor.matmul(qa[rc][:],
                                                 wq_s[:, hc, rc * 128:(rc + 1) * 128],
                                                 xt_g[:, c], start=st_, stop=sp_)
                            nc.tensor.matmul(ka[:], wk_s[:, hc], xt_g[:, c],
                                             start=st_, stop=sp_)
                            nc.tensor.matmul(vta[:], wv_s[:, hc], xt_g[:, c],
                                             start=st_, stop=sp_)
                    load_tail_weights()
                    # vt_s copy first on ACT so the V transposes (below,
                    # after the wo slice) never wait on the rotary copies
                    vt_s = rp.tile([128, SB], BF16, tag="vts")
                    nc.scalar.activation(vt_s[:], vta[:], COPY)
                    rotary(ka, s0, ccb, ssb, kT[:, s0:s0 + SB])
                    for rc in range(QH):
                        rotary(qa[rc], s0, ccb, ssb, qT[:, rc, s0:s0 + SB])

                    # previous block's first wo slice: independent PE work
                    # that covers the rotary-chain latency on ACT/DVE; the
                    # rest is interleaved into the attention head loop below
                    prev = pending_wo.pop() if pending_wo else None
                    if prev is not None:
                        emit_wo(*prev, 0, 8)

                    # vT [d, s] -> natural v j-tiles via PE transpose (bf16)
                    for t in range(SB // 128):
                        tp = ps.tile([128, 128], BF16, tag="ps", name="tp",
                                     padded_shape=[128, IB * 2])
                        nc.tensor.transpose(tp[:], vt_s[:, t * 128:(t + 1) * 128],
                                            id_s[:])
                        nc.scalar.activation(vn[:, (SB // 128) * sb + t], tp[:],
                                             COPY)

                    # ---- attention for i block ib == sb ----
                    ib = sb
                    i0 = ib * IB
                    njt = (ib + 1) * (IB // 128)
                    at = ap_.tile([128, QH, IB], BF16, tag="at")
                    for h in range(QH):
                        pv = ps.tile([128, IB], F32, tag="ps")
                        sumb = ps.tile([128, IB], F32, tag="ps")
                        pend = []
                        pbs = []
                        sums_ops = []

                        def flush(stop):
                            pjt, pc0, ppb = pend.pop(0)
                            nc.tensor.matmul(pv[:, pc0:], vn[:, pjt],
                                             ppb[:, pc0:],
                                             start=(pjt == 0), stop=stop)
                        for jt in range(njt):
                            kd = jt - ib * 4
                            c0 = kd * 128 if kd > 0 else 0
                            sc = ps.tile([128, IB], F32, tag=f"sc{jt % 2}",
                                         name="sc", bufs=1)
                            nc.tensor.matmul(sc[:, c0:],
                                             kT[:, jt * 128:(jt + 1) * 128],
                                             qT[:, h, i0 + c0:i0 + IB],
                                             start=True, stop=True)
                            pb = pp.tile([128, IB], BF16, tag=f"pb{jt}",
                                         bufs=1)
                            nc.scalar.activation(pb[:, c0:], sc[:, c0:], EXP)
                            if kd >= 0:
                                nc.vector.tensor_mul(pb[:, c0:c0 + 128],
                                                     pb[:, c0:c0 + 128],
                                                     tm_s[:])
                            pend.append((jt, c0, pb))
                            pbs.append(pb)
                            sums_ops.append((c0, pb))
                            if len(pend) > 2:
                                flush(False)
                        while pend:
                            flush(stop=(len(pend) == 1))
                        # softmax denominators: all-ones stationary sums pb
                        # over j, broadcast to all 128 partitions.  Pure PE
                        # work, scheduled after PV so it fills exp bubbles.
                        for k, (c0, src) in enumerate(sums_ops):
                            nc.tensor.matmul(sumb[:, c0:], ones_s[:],
                                             src[:, c0:],
                                             start=(k == 0),
                                             stop=(k == len(sums_ops) - 1))
                        rb = sg.tile([128, IB], F32, tag=f"rb{h % 2}", bufs=1)
                        nc.vector.reciprocal_approx_fast(rb[:], sumb[:])
                        nc.vector.tensor_mul(at[:, h], pv[:], rb[:])
                        # next slice of the previous block's wo: PE slack
                        # work covering this head's recip/normalize chain
                        if prev is not None:
                            emit_wo(*prev, 8 * (h + 1), 8 * (h + 2))
                    pending_wo.append((b, ib, at))

            emit_wo(*pending_wo.pop(), 0, H // 128)

    nc.compile()
    return nc


def _prep(x, freqs_cos, freqs_sin, wq, wk, wv, wo):
    """Shard + pre-transpose inputs for the 8 cores."""
    # rotary pair permutation: within each 32-partition quadrant, x0 of
    # pairs [16q,16q+16) sits in local slots 0..15 and x1 in 16..31.
    perm = np.zeros(HD, dtype=np.int64)
    pair = np.zeros(128, dtype=np.int64)
    sign = np.zeros(128, dtype=np.float32)
    for q in range(4):
        for t in range(16):
            perm[32 * q + t] = 2 * (16 * q + t)
            perm[32 * q + 16 + t] = 2 * (16 * q + t) + 1
            pair[32 * q + t] = 16 * q + t
            pair[32 * q + 16 + t] = 16 * q + t
            sign[32 * q + t] = -1.0
            sign[32 * q + 16 + t] = 1.0

    xt = np.ascontiguousarray(x.transpose(0, 2, 1)).astype(BF16NP)  # [B,H,S]
    cc = np.ascontiguousarray(freqs_cos[:, :, pair].transpose(0, 2, 1)).astype(BF16NP)
    ss2 = np.ascontiguousarray((freqs_sin[:, :, pair] * sign).transpose(0, 2, 1)).astype(BF16NP)

    # causal mask for the 128-wide boundary sub-tile of a diagonal j tile
    jj = np.arange(128)[:, None]
    ii = np.arange(128)[None, :]
    trimask = (jj <= ii).astype(BF16NP)

    scale = np.float32(1.0 / np.sqrt(HD))
    in_maps = []
    for c in range(NCORES):
        wq_c = (wq[c * RQ:(c + 1) * RQ] * scale).reshape(QH, HD, H)[:, perm, :]
        wqt = np.ascontiguousarray(wq_c.reshape(RQ, H).T).astype(BF16NP)
        wk_c = wk[c * HD:(c + 1) * HD][perm, :]
        wkt = np.ascontiguousarray(wk_c.T).astype(BF16NP)
        wvt = np.ascontiguousarray(wv[c * HD:(c + 1) * HD].T).astype(BF16NP)
        wot = np.ascontiguousarray(wo[:, c * RQ:(c + 1) * RQ].T).astype(BF16NP)
        in_maps.append({
            "xt": xt, "cc": cc, "ss2": ss2,
            "wqt": wqt, "wkt": wkt, "wvt": wvt, "wot": wot,
            "trimask": trimask, "ident": np.eye(128, dtype=BF16NP),
        })
    return in_maps


def kernel(x, freqs_cos, freqs_sin, wq, wk, wv, wo):
    global _CACHED, LAST_EXEC_NS
    x = np.asarray(x, dtype=np.float32)
    freqs_cos = np.asarray(freqs_cos, dtype=np.float32)
    freqs_sin = np.asarray(freqs_sin, dtype=np.float32)
    wq = np.asarray(wq, dtype=np.float32)
    wk = np.asarray(wk, dtype=np.float32)
    wv = np.asarray(wv, dtype=np.float32)
    wo = np.asarray(wo, dtype=np.float32)

    if _CACHED is None:
        _CACHED = _build()
    nc = _CACHED

    in_maps = _prep(x, freqs_cos, freqs_sin, wq, wk, wv, wo)
    res = run_bass_kernel_spmd(nc, in_maps, core_ids=list(range(NCORES)))
    LAST_EXEC_NS = res.exec_time_ns

    # unshard: sum the 8 partial wo products, then [B,H,S] -> [B,S,H]
    acc = res.results[0]["out"].astype(np.float64)
    for c in range(1, NCORES):
        acc += res.results[c]["out"]
    return np.ascontiguousarray(acc.transpose(0, 2, 1)).astype(np.float32)


# revision 20
# speedup vs baseline: 1.1730x; 1.0019x over previous
"""Multi-head GQA attention (B=2, S=2048, H=4096, 32 q-heads / 8 kv-heads,
HD=128, rotary, causal) on 8 TRN2 NeuronCores.

Sharding: tensor-parallel over heads, 8-way — core c owns q-heads
[4c, 4c+4) and kv-head c; wq/wk/wv column-sharded, wo row-sharded.  Each
core computes a partial wo product over its head slice for both batches;
the host sums the 8 partials (the TP reduction) and transposes back.

All on-device dataflow is in transposed layout [feature, seq] so no
device-side transposes are needed; the host pre-transposes x and the
weight shards during sharding.  Rotary pairs are laid out so the (x0,x1)
pair swap is a 32-partition-quadrant stream_shuffle on the Vector engine.

Engine assignment keeps the in-order ACT/DVE queues off the PE's
critical path: GpSimd issues all steady-state DMAs (25ns vs ~600ns on
SP/ACT) and takes half the wo-output PSUM->SBUF copies plus the V
transpose copies; ACT only does rotary PSUM copies and exp; DVE does
rotary arithmetic (bf16, 2x mode), masking, normalization, and the other
half of the wo copies.  Softmax denominators come from an all-ones
[128,128] stationary matmul accumulating a broadcast [128,512] sum
directly (no [1,512] row, no separate broadcast matmul), emitted as a
pure-PE pass after the PV accumulation.  Fully-masked regions of
diagonal blocks are skipped in the scores/exp/PV/sum pipeline.
"""
import sys

if "/opt/trn_rl_repo" not in sys.path:
    sys.path.insert(0, "/opt/trn_rl_repo")

import numpy as np
import ml_dtypes

from concourse import bacc, tile, mybir
from concourse.bass_utils import run_bass_kernel_spmd

F32 = mybir.dt.float32
BF16 = mybir.dt.bfloat16
EXP = mybir.ActivationFunctionType.Exp
COPY = mybir.ActivationFunctionType.Copy
BF16NP = ml_dtypes.bfloat16

B, S, H = 2, 2048, 4096
NH, NKV, HD = 32, 8, 128
NCORES = 8
QH = NH // NCORES          # 4 q-heads per core
RQ = QH * HD               # 512 q rows per core
SB = 512                   # projection seq block
NSB = S // SB              # 4
IB = 512                   # attention i block
NIB = S // IB              # 4
HC = H // 128              # 32 contraction chunks
NJT = S // 128             # 16 j tiles

# stream_shuffle mask: swap 16-partition halves within each 32-partition quadrant
SHUF = list(range(16, 32)) + list(range(16))

LAST_EXEC_NS = None
_CACHED = None


def _build(dbg=False):
    nc = bacc.Bacc("TRN2", target_bir_lowering=False, debug=False,
                   num_devices=NCORES)

    xt_e = nc.dram_tensor("xt", [B, H, S], BF16, kind="ExternalInput")
    cc_e = nc.dram_tensor("cc", [B, 128, S], BF16, kind="ExternalInput")
    ss_e = nc.dram_tensor("ss2", [B, 128, S], BF16, kind="ExternalInput")
    wq_e = nc.dram_tensor("wqt", [H, RQ], BF16, kind="ExternalInput")
    wk_e = nc.dram_tensor("wkt", [H, HD], BF16, kind="ExternalInput")
    wv_e = nc.dram_tensor("wvt", [H, HD], BF16, kind="ExternalInput")
    wo_e = nc.dram_tensor("wot", [RQ, H], BF16, kind="ExternalInput")
    tm_e = nc.dram_tensor("trimask", [128, 128], BF16, kind="ExternalInput")
    id_e = nc.dram_tensor("ident", [128, 128], BF16, kind="ExternalInput")
    out_e = nc.dram_tensor("out", [B, H, S], BF16, kind="ExternalOutput")

    with tile.TileContext(nc) as tc:
        with (nc.allow_low_precision(reason="bf16 compute by design"),
              tc.tile_pool(name="wpool", bufs=1) as wp,
              tc.tile_pool(name="state", bufs=1) as st,
              tc.tile_pool(name="att", bufs=2) as ap_,
              tc.tile_pool(name="xin", bufs=3) as xp,
              tc.tile_pool(name="probs", bufs=1) as pp,
              tc.tile_pool(name="rot", bufs=2) as rp,
              tc.tile_pool(name="stage", bufs=2) as sg,
              tc.tile_pool(name="ps", bufs=5, space="PSUM") as ps):

            # ---- resident weights ----
            wq_s = wp.tile([128, HC, RQ], BF16)
            wk_s = wp.tile([128, HC, HD], BF16)
            wv_s = wp.tile([128, HC, HD], BF16)
            wo_s = wp.tile([128, QH, H], BF16)
            tm_s = wp.tile([128, 128], BF16)
            ones_s = wp.tile([128, 128], BF16)
            id_s = wp.tile([128, 128], BF16)
            nc.vector.memset(ones_s[:], 1.0)

            # weight chunks on the Sync queue (startup only); x/cc/ss/out
            # on the GpSimd queue whose DMA issue is ~25ns
            _wchunk_loaded = [False] * (HC // 4)
            _tail_loaded = [False]

            def load_wchunk(hg):
                if _wchunk_loaded[hg]:
                    return
                _wchunk_loaded[hg] = True
                if hg == 0:
                    # single-chunk contiguous DMAs so the first matmul's
                    # dependencies land with minimal issue+transfer latency
                    nc.sync.dma_start(out=wq_s[:, 0], in_=wq_e.ap()[0:128])
                    nc.sync.dma_start(out=wk_s[:, 0], in_=wk_e.ap()[0:128])
                    nc.sync.dma_start(out=wv_s[:, 0], in_=wv_e.ap()[0:128])
                    return
                r0, r1 = hg * 512, (hg + 1) * 512
                nc.sync.dma_start(
                    out=wq_s[:, 4 * hg:4 * hg + 4],
                    in_=wq_e.ap()[r0:r1].rearrange("(c p) r -> p c r", c=4))
                nc.sync.dma_start(
                    out=wk_s[:, 4 * hg:4 * hg + 4],
                    in_=wk_e.ap()[r0:r1].rearrange("(c p) r -> p c r", c=4))
                nc.sync.dma_start(
                    out=wv_s[:, 4 * hg:4 * hg + 4],
                    in_=wv_e.ap()[r0:r1].rearrange("(c p) r -> p c r", c=4))

            def load_wchunk0_rest():
                # chunks 1-3, issued after the first x tile's DMA
                nc.sync.dma_start(
                    out=wq_s[:, 1:4],
                    in_=wq_e.ap()[128:512].rearrange("(c p) r -> p c r", c=3))
                nc.sync.dma_start(
                    out=wk_s[:, 1:4],
                    in_=wk_e.ap()[128:512].rearrange("(c p) r -> p c r", c=3))
                nc.sync.dma_start(
                    out=wv_s[:, 1:4],
                    in_=wv_e.ap()[128:512].rearrange("(c p) r -> p c r", c=3))

            def load_tail_weights():
                if _tail_loaded[0]:
                    return
                _tail_loaded[0] = True
                nc.sync.dma_start(out=id_s[:], in_=id_e.ap())
                nc.sync.dma_start(out=tm_s[:], in_=tm_e.ap())
                for rc in range(QH):
                    nc.sync.dma_start(out=wo_s[:, rc],
                                      in_=wo_e.ap()[rc * 128:(rc + 1) * 128])

            _rot_n = [0]

            def rotary(src_ps, s0, ccb, ssb, dst_ap):
                # dst = src*CC + shuffle(src)*SS2, bf16 (DVE 2x mode)
                r = _rot_n[0] = (_rot_n[0] + 1) % 2
                qs = rp.tile([128, SB], BF16, tag=f"qs{r}", bufs=1)
                nc.scalar.activation(qs[:], src_ps[:], COPY)
                qw = rp.tile([128, SB], BF16, tag=f"qw{r}", bufs=1)
                nc.vector.stream_shuffle(qw[:], qs[:], SHUF)
                nc.vector.tensor_mul(qs[:], qs[:], ccb[:, s0:s0 + SB])
                nc.vector.tensor_mul(qw[:], qw[:], ssb[:, s0:s0 + SB])
                nc.vector.tensor_add(dst_ap, qs[:], qw[:])

            pending_wo = []

            def emit_wo(wb, wib, wat, ho_lo, ho_hi):
                wi0 = wib * IB
                for ho in range(ho_lo, min(ho_hi, H // 128)):
                    acc = ps.tile([128, IB], F32, tag="ps", name="acc")
                    for rc in range(QH):
                        nc.tensor.matmul(acc[:],
                                         wo_s[:, rc, ho * 128:(ho + 1) * 128],
                                         wat[:, rc],
                                         start=(rc == 0), stop=(rc == QH - 1))
                    og = sg.tile([128, IB], BF16, tag=f"og{ho % 4}", name="og",
                                 bufs=1)
                    if ho % 2 == 0:
                        nc.scalar.activation(og[:], acc[:], COPY)
                    else:
                        nc.vector.tensor_copy(og[:], acc[:])
                    oeng = nc.scalar if ho % 2 == 0 else nc.sync
                    oeng.dma_start(
                        out=out_e.ap()[wb, ho * 128:(ho + 1) * 128, wi0:wi0 + IB],
                        in_=og[:])

            for b in range(B):
                qT = st.tile([128, QH, S], BF16, tag="qT")
                kT = st.tile([128, S], BF16, tag="kT")
                vn = st.tile([128, NJT, HD], BF16, tag="vn")
                ccb = st.tile([128, S], BF16, tag="cc")
                ssb = st.tile([128, S], BF16, tag="ss")

                # interleaved: projection block sb feeds attention block
                # ib == sb immediately (causal: ib needs only j <= (sb+1)*SB)
                for sb in range(NSB):
                    s0 = sb * SB
                    # ---- projection for seq block sb ----
                    qa = [ps.tile([128, SB], F32, tag="ps", name=f"qa{rc}")
                          for rc in range(QH)]
                    ka = ps.tile([128, SB], F32, tag="ps")
                    # vta lives in the "sc2" bank: the proj and attention
                    # phases are disjoint, so it never collides with scores
                    vta = ps.tile([128, SB], F32, tag="sc2", bufs=1)
                    for hg in range(HC // 4):
                        first = not _wchunk_loaded[hg] and hg == 0
                        load_wchunk(hg)
                        xt_g = xp.tile([128, 4, SB], BF16, tag=f"xt{hg % 4}",
                                       bufs=1)
                        if first:
                            # split the first x tile across two queues (two
                            # DMA engines) to halve its transfer latency
                            for c in range(4):
                                eng = nc.sync if c % 2 == 0 else nc.scalar
                                eng.dma_start(
                                    out=xt_g[:, c],
                                    in_=xt_e.ap()[b, c * 128:(c + 1) * 128,
                                                  s0:s0 + SB])
                            load_wchunk0_rest()
                        else:
                            eng = nc.sync if hg % 2 == 0 else nc.scalar
                            eng.dma_start(
                                out=xt_g[:],
                                in_=xt_e.ap()[b, hg * 512:(hg + 1) * 512,
                                              s0:s0 + SB]
                                .rearrange("(c p) s -> p c s", c=4))
                        if sb == 0 and hg == 1:
                            nc.scalar.dma_start(out=ccb[:], in_=cc_e.ap()[b])
                            nc.scalar.dma_start(out=ssb[:], in_=ss_e.ap()[b])
                        for c in range(4):
                            hc = hg * 4 + c
                            st_, sp_ = (hc == 0), (hc == HC - 1)
                            for rc in range(QH):
                                nc.tensor.matmul(qa[rc][:],
                                                 wq_s[:, hc, rc * 128:(rc + 1) * 128],
                                                 xt_g[:, c], start=st_, stop=sp_)
                            nc.tensor.matmul(ka[:], wk_s[:, hc], xt_g[:, c],
                                             start=st_, stop=sp_)
                            nc.tensor.matmul(vta[:], wv_s[:, hc], xt_g[:, c],
                                             start=st_, stop=sp_)
                    load_tail_weights()
                    # vt_s copy first on ACT so the V transposes (below,
                    # after the wo slice) never wait on the rotary copies
                    vt_s = rp.tile([128, SB], BF16, tag="vts")
                    nc.scalar.activation(vt_s[:], vta[:], COPY)
                    rotary(ka, s0, ccb, ssb, kT[:, s0:s0 + SB])
                    for rc in range(QH):
                        rotary(qa[rc], s0, ccb, ssb, qT[:, rc, s0:s0 + SB])

                    # previous block's first wo slice: independent PE work
                    # that covers the rotary-chain latency on ACT/DVE; the
                    # rest is interleaved into the attention head loop below
                    prev = pending_wo.pop() if pending_wo else None
                    if prev is not None:
                        emit_wo(*prev, 0, 8)

                    # vT [d, s] -> natural v j-tiles via PE transpose (bf16)
                    for t in range(SB // 128):
                        tp = ps.tile([128, 128], BF16, tag="ps", name="tp",
                                     padded_shape=[128, IB * 2])
                        nc.tensor.transpose(tp[:], vt_s[:, t * 128:(t + 1) * 128],
                                            id_s[:])
                        nc.scalar.activation(vn[:, (SB // 128) * sb + t], tp[:],
                                             COPY)

                    # ---- attention for i block ib == sb ----
                    ib = sb
                    i0 = ib * IB
                    njt = (ib + 1) * (IB // 128)
                    at = ap_.tile([128, QH, IB], BF16, tag="at")
                    for h in range(QH):
                        pv = ps.tile([128, IB], F32, tag="ps")
                        sumb = ps.tile([128, IB], F32, tag="ps")
                        pend = []
                        pbs = []
                        sums_ops = []

                        def flush(stop):
                            pjt, pc0, ppb = pend.pop(0)
                            nc.tensor.matmul(pv[:, pc0:], vn[:, pjt],
                                             ppb[:, pc0:],
                                             start=(pjt == 0), stop=stop)
                        for jt in range(njt):
                            kd = jt - ib * 4
                            c0 = kd * 128 if kd > 0 else 0
                            sc = ps.tile([128, IB], F32, tag=f"sc{jt % 3}",
                                         name="sc", bufs=1)
                            nc.tensor.matmul(sc[:, c0:],
                                             kT[:, jt * 128:(jt + 1) * 128],
                                             qT[:, h, i0 + c0:i0 + IB],
                                             start=True, stop=True)
                            pb = pp.tile([128, IB], BF16, tag=f"pb{jt}",
                                         bufs=1)
                            nc.scalar.activation(pb[:, c0:], sc[:, c0:], EXP)
                            if kd >= 0:
                                nc.vector.tensor_mul(pb[:, c0:c0 + 128],
                                                     pb[:, c0:c0 + 128],
                                                     tm_s[:])
                            pend.append((jt, c0, pb))
                            pbs.append(pb)
                            sums_ops.append((c0, pb))
                            if len(pend) > 2:
                                flush(False)
                        while pend:
                            flush(stop=(len(pend) == 1))
                        # softmax denominators: all-ones stationary sums pb
                        # over j, broadcast to all 128 partitions.  Pure PE
                        # work, scheduled after PV so it fills exp bubbles.
                        for k, (c0, src) in enumerate(sums_ops):
                            nc.tensor.matmul(sumb[:, c0:], ones_s[:],
                                             src[:, c0:],
                                             start=(k == 0),
                                             stop=(k == len(sums_ops) - 1))
                        rb = sg.tile([128, IB], F32, tag=f"rb{h % 2}", bufs=1)
                        nc.vector.reciprocal_approx_fast(rb[:], sumb[:])
                        nc.vector.tensor_mul(at[:, h], pv[:], rb[:])
                        # next slice of the previous block's wo: PE slack
                        # work covering this head's recip/normalize chain
                        if prev is not None:
                            emit_wo(*prev, 8 * (h + 1), 8 * (h + 2))
                    pending_wo.append((b, ib, at))

            emit_wo(*pending_wo.pop(), 0, H // 128)

    nc.compile()
    return nc


def _prep(x, freqs_cos, freqs_sin, wq, wk, wv, wo):
    """Shard + pre-transpose inputs for the 8 cores."""
    # rotary pair permutation: within each 32-partition quadrant, x0 of
    # pairs [16q,16q+16) sits in local slots 0..15 and x1 in 16..31.
    perm = np.zeros(HD, dtype=np.int64)
    pair = np.zeros(128, dtype=np.int64)
    sign = np.zeros(128, dtype=np.float32)
    for q in range(4):
        for t in range(16):
            perm[32 * q + t] = 2 * (16 * q + t)
            perm[32 * q + 16 + t] = 2 * (16 * q + t) + 1
            pair[32 * q + t] = 16 * q + t
            pair[32 * q + 16 + t] = 16 * q + t
            sign[32 * q + t] = -1.0
            sign[32 * q + 16 + t] = 1.0

    xt = np.ascontiguousarray(x.transpose(0, 2, 1)).astype(BF16NP)  # [B,H,S]
    cc = np.ascontiguousarray(freqs_cos[:, :, pair].transpose(0, 2, 1)).astype(BF16NP)
    ss2 = np.ascontiguousarray((freqs_sin[:, :, pair] * sign).transpose(0, 2, 1)).astype(BF16NP)

    # causal mask for the 128-wide boundary sub-tile of a diagonal j tile
    jj = np.arange(128)[:, None]
    ii = np.arange(128)[None, :]
    trimask = (jj <= ii).astype(BF16NP)

    scale = np.float32(1.0 / np.sqrt(HD))
    in_maps = []
    for c in range(NCORES):
        wq_c = (wq[c * RQ:(c + 1) * RQ] * scale).reshape(QH, HD, H)[:, perm, :]
        wqt = np.ascontiguousarray(wq_c.reshape(RQ, H).T).astype(BF16NP)
        wk_c = wk[c * HD:(c + 1) * HD][perm, :]
        wkt = np.ascontiguousarray(wk_c.T).astype(BF16NP)
        wvt = np.ascontiguousarray(wv[c * HD:(c + 1) * HD].T).astype(BF16NP)
        wot = np.ascontiguousarray(wo[:, c * RQ:(c + 1) * RQ].T).astype(BF16NP)
        in_maps.append({
            "xt": xt, "cc": cc, "ss2": ss2,
            "wqt": wqt, "wkt": wkt, "wvt": wvt, "wot": wot,
            "trimask": trimask, "ident": np.eye(128, dtype=BF16NP),
        })
    return in_maps


def kernel(x, freqs_cos, freqs_sin, wq, wk, wv, wo):
    global _CACHED, LAST_EXEC_NS
    x = np.asarray(x, dtype=np.float32)
    freqs_cos = np.asarray(freqs_cos, dtype=np.float32)
    freqs_sin = np.asarray(freqs_sin, dtype=np.float32)
    wq = np.asarray(wq, dtype=np.float32)
    wk = np.asarray(wk, dtype=np.float32)
    wv = np.asarray(wv, dtype=np.float32)
    wo = np.asarray(wo, dtype=np.float32)

    if _CACHED is None:
        _CACHED = _build()
    nc = _CACHED

    in_maps = _prep(x, freqs_cos, freqs_sin, wq, wk, wv, wo)
    res = run_bass_kernel_spmd(nc, in_maps, core_ids=list(range(NCORES)))
    LAST_EXEC_NS = res.exec_time_ns

    # unshard: sum the 8 partial wo products, then [B,H,S] -> [B,S,H]
    acc = res.results[0]["out"].astype(np.float64)
    for c in range(1, NCORES):
        acc += res.results[c]["out"]
    return np.ascontiguousarray(acc.transpose(0, 2, 1)).astype(np.float32)


# revision 24
# speedup vs baseline: 1.1889x; 1.0136x over previous
"""Multi-head GQA attention (B=2, S=2048, H=4096, 32 q-heads / 8 kv-heads,
HD=128, rotary, causal) on 8 TRN2 NeuronCores.

Sharding: tensor-parallel over heads, 8-way — core c owns q-heads
[4c, 4c+4) and kv-head c; wq/wk/wv column-sharded, wo row-sharded.  Each
core computes a partial wo product over its head slice for both batches;
the host sums the 8 partials (the TP reduction) and transposes back.

All on-device dataflow is in transposed layout [feature, seq] so no
device-side transposes are needed; the host pre-transposes x and the
weight shards during sharding.  Rotary pairs are laid out so the (x0,x1)
pair swap is a 32-partition-quadrant stream_shuffle on the Vector engine.

Engine assignment keeps the in-order ACT/DVE queues off the PE's
critical path: GpSimd issues all steady-state DMAs (25ns vs ~600ns on
SP/ACT) and takes half the wo-output PSUM->SBUF copies plus the V
transpose copies; ACT only does rotary PSUM copies and exp; DVE does
rotary arithmetic (bf16, 2x mode), masking, normalization, and the other
half of the wo copies.  Softmax denominators come from an all-ones
[128,128] stationary matmul accumulating a broadcast [128,512] sum
directly (no [1,512] row, no separate broadcast matmul), emitted as a
pure-PE pass after the PV accumulation.  Fully-masked regions of
diagonal blocks are skipped in the scores/exp/PV/sum pipeline.
"""
import sys

if "/opt/trn_rl_repo" not in sys.path:
    sys.path.insert(0, "/opt/trn_rl_repo")

import numpy as np
import ml_dtypes

from concourse import bacc, tile, mybir
from concourse.bass_utils import run_bass_kernel_spmd

F32 = mybir.dt.float32
BF16 = mybir.dt.bfloat16
EXP = mybir.ActivationFunctionType.Exp
COPY = mybir.ActivationFunctionType.Copy
BF16NP = ml_dtypes.bfloat16

B, S, H = 2, 2048, 4096
NH, NKV, HD = 32, 8, 128
NCORES = 8
QH = NH // NCORES          # 4 q-heads per core
RQ = QH * HD               # 512 q rows per core
SB = 512                   # projection seq block
NSB = S // SB              # 4
IB = 512                   # attention i block
NIB = S // IB              # 4
HC = H // 128              # 32 contraction chunks
NJT = S // 128             # 16 j tiles

# stream_shuffle mask: swap 16-partition halves within each 32-partition quadrant
SHUF = list(range(16, 32)) + list(range(16))

LAST_EXEC_NS = None
_CACHED = None


def _build(dbg=False):
    nc = bacc.Bacc("TRN2", target_bir_lowering=False, debug=False,
                   num_devices=NCORES)

    xt_e = nc.dram_tensor("xt", [B, H, S], BF16, kind="ExternalInput")
    cc_e = nc.dram_tensor("cc", [B, 128, S], BF16, kind="ExternalInput")
    ss_e = nc.dram_tensor("ss2", [B, 128, S], BF16, kind="ExternalInput")
    wq_e = nc.dram_tensor("wqt", [H, RQ], BF16, kind="ExternalInput")
    wk_e = nc.dram_tensor("wkt", [H, HD], BF16, kind="ExternalInput")
    wv_e = nc.dram_tensor("wvt", [H, HD], BF16, kind="ExternalInput")
    wo_e = nc.dram_tensor("wot", [RQ, H], BF16, kind="ExternalInput")
    tm_e = nc.dram_tensor("trimask", [128, 128], BF16, kind="ExternalInput")
    id_e = nc.dram_tensor("ident", [128, 128], BF16, kind="ExternalInput")
    out_e = nc.dram_tensor("out", [B, H, S], BF16, kind="ExternalOutput")

    with tile.TileContext(nc) as tc:
        with (nc.allow_low_precision(reason="bf16 compute by design"),
              tc.tile_pool(name="wpool", bufs=1) as wp,
              tc.tile_pool(name="state", bufs=1) as st,
              tc.tile_pool(name="att", bufs=2) as ap_,
              tc.tile_pool(name="xin", bufs=3) as xp,
              tc.tile_pool(name="probs", bufs=1) as pp,
              tc.tile_pool(name="rot", bufs=2) as rp,
              tc.tile_pool(name="stage", bufs=2) as sg,
              tc.tile_pool(name="ps", bufs=5, space="PSUM") as ps):

            # ---- resident weights ----
            wq_s = wp.tile([128, HC, RQ], BF16)
            wk_s = wp.tile([128, HC, HD], BF16)
            wv_s = wp.tile([128, HC, HD], BF16)
            wo_s = wp.tile([128, QH, H], BF16)
            tm_s = wp.tile([128, 128], BF16)
            ones_s = wp.tile([128, 128], BF16)
            id_s = wp.tile([128, 128], BF16)
            nc.vector.memset(ones_s[:], 1.0)

            # weight chunks on the Sync queue (startup only); x/cc/ss/out
            # on the GpSimd queue whose DMA issue is ~25ns
            _wchunk_loaded = [False] * (HC // 4)
            _tail_loaded = [False]

            def load_wchunk(hg):
                if _wchunk_loaded[hg]:
                    return
                _wchunk_loaded[hg] = True
                if hg == 0:
                    # single-chunk contiguous DMAs so the first matmul's
                    # dependencies land with minimal issue+transfer latency
                    nc.sync.dma_start(out=wq_s[:, 0], in_=wq_e.ap()[0:128])
                    nc.sync.dma_start(out=wk_s[:, 0], in_=wk_e.ap()[0:128])
                    nc.sync.dma_start(out=wv_s[:, 0], in_=wv_e.ap()[0:128])
                    return
                r0, r1 = hg * 512, (hg + 1) * 512
                nc.sync.dma_start(
                    out=wq_s[:, 4 * hg:4 * hg + 4],
                    in_=wq_e.ap()[r0:r1].rearrange("(c p) r -> p c r", c=4))
                nc.sync.dma_start(
                    out=wk_s[:, 4 * hg:4 * hg + 4],
                    in_=wk_e.ap()[r0:r1].rearrange("(c p) r -> p c r", c=4))
                nc.sync.dma_start(
                    out=wv_s[:, 4 * hg:4 * hg + 4],
                    in_=wv_e.ap()[r0:r1].rearrange("(c p) r -> p c r", c=4))

            def load_wchunk0_rest():
                # chunks 1-3, issued after the first x tile's DMA
                nc.sync.dma_start(
                    out=wq_s[:, 1:4],
                    in_=wq_e.ap()[128:512].rearrange("(c p) r -> p c r", c=3))
                nc.sync.dma_start(
                    out=wk_s[:, 1:4],
                    in_=wk_e.ap()[128:512].rearrange("(c p) r -> p c r", c=3))
                nc.sync.dma_start(
                    out=wv_s[:, 1:4],
                    in_=wv_e.ap()[128:512].rearrange("(c p) r -> p c r", c=3))

            def load_tail_weights():
                if _tail_loaded[0]:
                    return
                _tail_loaded[0] = True
                nc.sync.dma_start(out=id_s[:], in_=id_e.ap())
                nc.sync.dma_start(out=tm_s[:], in_=tm_e.ap())
                for rc in range(QH):
                    nc.sync.dma_start(out=wo_s[:, rc],
                                      in_=wo_e.ap()[rc * 128:(rc + 1) * 128])

            _rot_n = [0]

            def rotary(src_ps, s0, ccb, ssb, dst_ap):
                # dst = src*CC + shuffle(src)*SS2, bf16 (DVE 2x mode)
                r = _rot_n[0] = (_rot_n[0] + 1) % 2
                qs = rp.tile([128, SB], BF16, tag=f"qs{r}", bufs=1)
                nc.scalar.activation(qs[:], src_ps[:], COPY)
                qw = rp.tile([128, SB], BF16, tag=f"qw{r}", bufs=1)
                nc.vector.stream_shuffle(qw[:], qs[:], SHUF)
                nc.vector.tensor_mul(qs[:], qs[:], ccb[:, s0:s0 + SB])
                nc.vector.tensor_mul(qw[:], qw[:], ssb[:, s0:s0 + SB])
                nc.vector.tensor_add(dst_ap, qs[:], qw[:])

            pending_wo = []

            og_grp = [None]

            def emit_wo(wb, wib, wat, ho_lo, ho_hi):
                # 4 ho chunks share one og tile and one grouped output DMA
                wi0 = wib * IB
                for ho in range(ho_lo, min(ho_hi, H // 128)):
                    acc = ps.tile([128, IB], F32, tag="ps", name="acc")
                    for rc in range(QH):
                        nc.tensor.matmul(acc[:],
                                         wo_s[:, rc, ho * 128:(ho + 1) * 128],
                                         wat[:, rc],
                                         start=(rc == 0), stop=(rc == QH - 1))
                    if ho % 4 == 0:
                        og_grp[0] = sg.tile([128, 4, IB], BF16,
                                            tag=f"og{(ho // 4) % 2}",
                                            name="og", bufs=1)
                    og = og_grp[0]
                    if ho % 2 == 0:
                        nc.scalar.activation(og[:, ho % 4], acc[:], COPY)
                    else:
                        nc.vector.tensor_copy(og[:, ho % 4], acc[:])
                    if ho % 4 == 3:
                        oeng = nc.scalar if (ho // 4) % 2 == 0 else nc.sync
                        oeng.dma_start(
                            out=out_e.ap()[wb, (ho - 3) * 128:(ho + 1) * 128,
                                           wi0:wi0 + IB]
                            .rearrange("(c p) s -> p c s", c=4),
                            in_=og[:])

            for b in range(B):
                qT = st.tile([128, QH, S], BF16, tag="qT")
                kT = st.tile([128, S], BF16, tag="kT")
                vn = st.tile([128, NJT, HD], BF16, tag="vn")
                ccb = st.tile([128, S], BF16, tag="cc")
                ssb = st.tile([128, S], BF16, tag="ss")

                # interleaved: projection block sb feeds attention block
                # ib == sb immediately (causal: ib needs only j <= (sb+1)*SB)
                for sb in range(NSB):
                    s0 = sb * SB
                    # ---- projection for seq block sb ----
                    qa = [ps.tile([128, SB], F32, tag="ps", name=f"qa{rc}")
                          for rc in range(QH)]
                    ka = ps.tile([128, SB], F32, tag="ps")
                    # vta lives in the "sc2" bank: the proj and attention
                    # phases are disjoint, so it never collides with scores
                    vta = ps.tile([128, SB], F32, tag="sc2", bufs=1)
                    for hg in range(HC // 4):
                        first = not _wchunk_loaded[hg] and hg == 0
                        load_wchunk(hg)
                        xt_g = xp.tile([128, 4, SB], BF16, tag=f"xt{hg % 4}",
                                       bufs=1)
                        if b == 0 and sb == 0 and hg <= 1:
                            # split the first x tiles across two queues (two
                            # DMA engines) to halve their transfer latency
                            for c in range(4):
                                eng = nc.sync if c % 2 == 0 else nc.scalar
                                eng.dma_start(
                                    out=xt_g[:, c],
                                    in_=xt_e.ap()[b,
                                                  hg * 512 + c * 128:
                                                  hg * 512 + (c + 1) * 128,
                                                  s0:s0 + SB])
                        else:
                            eng = nc.sync if hg % 2 == 0 else nc.scalar
                            eng.dma_start(
                                out=xt_g[:],
                                in_=xt_e.ap()[b, hg * 512:(hg + 1) * 512,
                                              s0:s0 + SB]
                                .rearrange("(c p) s -> p c s", c=4))
                        if first:
                            load_wchunk0_rest()
                        if sb == 0 and hg == 3:
                            nc.scalar.dma_start(out=ccb[:], in_=cc_e.ap()[b])
                            nc.sync.dma_start(out=ssb[:], in_=ss_e.ap()[b])
                        for c in range(4):
                            hc = hg * 4 + c
                            st_, sp_ = (hc == 0), (hc == HC - 1)
                            for rc in range(QH):
                                nc.tensor.matmul(qa[rc][:],
                                                 wq_s[:, hc, rc * 128:(rc + 1) * 128],
                                                 xt_g[:, c], start=st_, stop=sp_)
                            nc.tensor.matmul(ka[:], wk_s[:, hc], xt_g[:, c],
                                             start=st_, stop=sp_)
                            nc.tensor.matmul(vta[:], wv_s[:, hc], xt_g[:, c],
                                             start=st_, stop=sp_)
                    load_tail_weights()
                    # vt_s copy first on ACT so the V transposes (below,
                    # after the wo slice) never wait on the rotary copies
                    vt_s = rp.tile([128, SB], BF16, tag="vts")
                    nc.scalar.activation(vt_s[:], vta[:], COPY)
                    rotary(ka, s0, ccb, ssb, kT[:, s0:s0 + SB])
                    for rc in range(QH):
                        rotary(qa[rc], s0, ccb, ssb, qT[:, rc, s0:s0 + SB])

                    # previous block's first wo slice: independent PE work
                    # that covers the rotary-chain latency on ACT/DVE; the
                    # rest is interleaved into the attention head loop below
                    prev = pending_wo.pop() if pending_wo else None
                    if prev is not None:
                        emit_wo(*prev, 0, 8)

                    # vT [d, s] -> natural v j-tiles via PE transpose (bf16)
                    for t in range(SB // 128):
                        tp = ps.tile([128, 128], BF16, tag="ps", name="tp",
                                     padded_shape=[128, IB * 2])
                        nc.tensor.transpose(tp[:], vt_s[:, t * 128:(t + 1) * 128],
                                            id_s[:])
                        nc.scalar.activation(vn[:, (SB // 128) * sb + t], tp[:],
                                             COPY)

                    # ---- attention for i block ib == sb ----
                    ib = sb
                    i0 = ib * IB
                    njt = (ib + 1) * (IB // 128)
                    at = ap_.tile([128, QH, IB], BF16, tag="at")
                    for h in range(QH):
                        pv = ps.tile([128, IB], F32, tag="ps")
                        sumb = ps.tile([128, IB], F32, tag="ps")
                        pend = []
                        pbs = []
                        sums_ops = []

                        def flush(stop):
                            pjt, pc0, ppb = pend.pop(0)
                            nc.tensor.matmul(pv[:, pc0:], vn[:, pjt],
                                             ppb[:, pc0:],
                                             start=(pjt == 0), stop=stop)
                        for jt in range(njt):
                            kd = jt - ib * 4
                            c0 = kd * 128 if kd > 0 else 0
                            sc = ps.tile([128, IB], F32, tag=f"sc{jt % 3}",
                                         name="sc", bufs=1)
                            nc.tensor.matmul(sc[:, c0:],
                                             kT[:, jt * 128:(jt + 1) * 128],
                                             qT[:, h, i0 + c0:i0 + IB],
                                             start=True, stop=True)
                            if kd >= 0:
                                # causal mask: accumulate a -60 upper triangle
                                # into the boundary 128 columns on the PE, so
                                # exp zeroes it — no DVE op, no mask->PV dep
                                nc.tensor.matmul(sc[:, c0:c0 + 128], id_s[:],
                                                 tm_s[:], start=False,
                                                 stop=True,
                                                 skip_group_check=True)
                            pb = pp.tile([128, IB], BF16, tag=f"pb{jt}",
                                         bufs=1)
                            nc.scalar.activation(pb[:, c0:], sc[:, c0:], EXP)
                            pend.append((jt, c0, pb))
                            pbs.append(pb)
                            sums_ops.append((c0, pb))
                            if len(pend) > 2:
                                flush(False)
                        while pend:
                            flush(stop=(len(pend) == 1))
                        # softmax denominators: all-ones stationary sums pb
                        # over j, broadcast to all 128 partitions.  Pure PE
                        # work, scheduled after PV so it fills exp bubbles.
                        for k, (c0, src) in enumerate(sums_ops):
                            nc.tensor.matmul(sumb[:, c0:], ones_s[:],
                                             src[:, c0:],
                                             start=(k == 0),
                                             stop=(k == len(sums_ops) - 1))
                        rb = sg.tile([128, IB], F32, tag=f"rb{h % 2}", bufs=1)
                        nc.vector.reciprocal_approx_fast(rb[:], sumb[:])
                        nc.vector.tensor_mul(at[:, h], pv[:], rb[:])
                        # next slice of the previous block's wo: PE slack
                        # work covering this head's recip/normalize chain
                        if prev is not None:
                            emit_wo(*prev, 8 * (h + 1), 8 * (h + 2))
                    pending_wo.append((b, ib, at))

            emit_wo(*pending_wo.pop(), 0, H // 128)

    nc.compile()
    return nc


def _prep(x, freqs_cos, freqs_sin, wq, wk, wv, wo):
    """Shard + pre-transpose inputs for the 8 cores."""
    # rotary pair permutation: within each 32-partition quadrant, x0 of
    # pairs [16q,16q+16) sits in local slots 0..15 and x1 in 16..31.
    perm = np.zeros(HD, dtype=np.int64)
    pair = np.zeros(128, dtype=np.int64)
    sign = np.zeros(128, dtype=np.float32)
    for q in range(4):
        for t in range(16):
            perm[32 * q + t] = 2 * (16 * q + t)
            perm[32 * q + 16 + t] = 2 * (16 * q + t) + 1
            pair[32 * q + t] = 16 * q + t
            pair[32 * q + 16 + t] = 16 * q + t
            sign[32 * q + t] = -1.0
            sign[32 * q + 16 + t] = 1.0

    xt = np.ascontiguousarray(x.transpose(0, 2, 1)).astype(BF16NP)  # [B,H,S]
    cc = np.ascontiguousarray(freqs_cos[:, :, pair].transpose(0, 2, 1)).astype(BF16NP)
    ss2 = np.ascontiguousarray((freqs_sin[:, :, pair] * sign).transpose(0, 2, 1)).astype(BF16NP)

    # additive causal mask for the 128-wide boundary sub-tile of a diagonal
    # j tile: 0 where j <= i, -60 above (exp(-60+s) == 0 in bf16/f32)
    jj = np.arange(128)[:, None]
    ii = np.arange(128)[None, :]
    trimask = np.where(jj <= ii, 0.0, -60.0).astype(BF16NP)

    scale = np.float32(1.0 / np.sqrt(HD))
    in_maps = []
    for c in range(NCORES):
        wq_c = (wq[c * RQ:(c + 1) * RQ] * scale).reshape(QH, HD, H)[:, perm, :]
        wqt = np.ascontiguousarray(wq_c.reshape(RQ, H).T).astype(BF16NP)
        wk_c = wk[c * HD:(c + 1) * HD][perm, :]
        wkt = np.ascontiguousarray(wk_c.T).astype(BF16NP)
        wvt = np.ascontiguousarray(wv[c * HD:(c + 1) * HD].T).astype(BF16NP)
        wot = np.ascontiguousarray(wo[:, c * RQ:(c + 1) * RQ].T).astype(BF16NP)
        in_maps.append({
            "xt": xt, "cc": cc, "ss2": ss2,
            "wqt": wqt, "wkt": wkt, "wvt": wvt, "wot": wot,
            "trimask": trimask, "ident": np.eye(128, dtype=BF16NP),
        })
    return in_maps


def kernel(x, freqs_cos, freqs_sin, wq, wk, wv, wo):
    global _CACHED, LAST_EXEC_NS
    x = np.asarray(x, dtype=np.float32)
    freqs_cos = np.asarray(freqs_cos, dtype=np.float32)
    freqs_sin = np.asarray(freqs_sin, dtype=np.float32)
    wq = np.asarray(wq, dtype=np.float32)
    wk = np.asarray(wk, dtype=np.float32)
    wv = np.asarray(wv, dtype=np.float32)
    wo = np.asarray(wo, dtype=np.float32)

    if _CACHED is None:
        _CACHED = _build()
    nc = _CACHED

    in_maps = _prep(x, freqs_cos, freqs_sin, wq, wk, wv, wo)
    res = run_bass_kernel_spmd(nc, in_maps, core_ids=list(range(NCORES)))
    LAST_EXEC_NS = res.exec_time_ns

    # unshard: sum the 8 partial wo products, then [B,H,S] -> [B,S,H]
    acc = res.results[0]["out"].astype(np.float64)
    for c in range(1, NCORES):
        acc += res.results[c]["out"]
    return np.ascontiguousarray(acc.transpose(0, 2, 1)).astype(np.float32)


# revision 26
# speedup vs baseline: 1.1914x; 1.0021x over previous
"""Multi-head GQA attention (B=2, S=2048, H=4096, 32 q-heads / 8 kv-heads,
HD=128, rotary, causal) on 8 TRN2 NeuronCores.

Sharding: tensor-parallel over heads, 8-way — core c owns q-heads
[4c, 4c+4) and kv-head c; wq/wk/wv column-sharded, wo row-sharded.  Each
core computes a partial wo product over its head slice for both batches;
the host sums the 8 partials (the TP reduction) and transposes back.

All on-device dataflow is in transposed layout [feature, seq] so no
device-side transposes are needed; the host pre-transposes x and the
weight shards during sharding.  Rotary pairs are laid out so the (x0,x1)
pair swap is a 32-partition-quadrant stream_shuffle on the Vector engine.

Engine assignment keeps the in-order ACT/DVE queues off the PE's
critical path: GpSimd issues all steady-state DMAs (25ns vs ~600ns on
SP/ACT) and takes half the wo-output PSUM->SBUF copies plus the V
transpose copies; ACT only does rotary PSUM copies and exp; DVE does
rotary arithmetic (bf16, 2x mode), masking, normalization, and the other
half of the wo copies.  Softmax denominators come from an all-ones
[128,128] stationary matmul accumulating a broadcast [128,512] sum
directly (no [1,512] row, no separate broadcast matmul), emitted as a
pure-PE pass after the PV accumulation.  Fully-masked regions of
diagonal blocks are skipped in the scores/exp/PV/sum pipeline.
"""
import sys

if "/opt/trn_rl_repo" not in sys.path:
    sys.path.insert(0, "/opt/trn_rl_repo")

import numpy as np
import ml_dtypes

from concourse import bacc, tile, mybir
from concourse.bass_utils import run_bass_kernel_spmd

F32 = mybir.dt.float32
BF16 = mybir.dt.bfloat16
EXP = mybir.ActivationFunctionType.Exp
COPY = mybir.ActivationFunctionType.Copy
BF16NP = ml_dtypes.bfloat16

B, S, H = 2, 2048, 4096
NH, NKV, HD = 32, 8, 128
NCORES = 8
QH = NH // NCORES          # 4 q-heads per core
RQ = QH * HD               # 512 q rows per core
SB = 512                   # projection seq block
NSB = S // SB              # 4
IB = 512                   # attention i block
NIB = S // IB              # 4
HC = H // 128              # 32 contraction chunks
NJT = S // 128             # 16 j tiles

# stream_shuffle mask: swap 16-partition halves within each 32-partition quadrant
SHUF = list(range(16, 32)) + list(range(16))

LAST_EXEC_NS = None
_CACHED = None


def _build(dbg=False):
    nc = bacc.Bacc("TRN2", target_bir_lowering=False, debug=False,
                   num_devices=NCORES)

    xt_e = nc.dram_tensor("xt", [B, H, S], BF16, kind="ExternalInput")
    cc_e = nc.dram_tensor("cc", [B, 128, S], BF16, kind="ExternalInput")
    ss_e = nc.dram_tensor("ss2", [B, 128, S], BF16, kind="ExternalInput")
    wq_e = nc.dram_tensor("wqt", [H, RQ], BF16, kind="ExternalInput")
    wk_e = nc.dram_tensor("wkt", [H, HD], BF16, kind="ExternalInput")
    wv_e = nc.dram_tensor("wvt", [H, HD], BF16, kind="ExternalInput")
    wo_e = nc.dram_tensor("wot", [RQ, H], BF16, kind="ExternalInput")
    tm_e = nc.dram_tensor("trimask", [128, 128], BF16, kind="ExternalInput")
    id_e = nc.dram_tensor("ident", [128, 128], BF16, kind="ExternalInput")
    out_e = nc.dram_tensor("out", [B, H, S], BF16, kind="ExternalOutput")

    with tile.TileContext(nc) as tc:
        with (nc.allow_low_precision(reason="bf16 compute by design"),
              tc.tile_pool(name="wpool", bufs=1) as wp,
              tc.tile_pool(name="state", bufs=1) as st,
              tc.tile_pool(name="att", bufs=2) as ap_,
              tc.tile_pool(name="xin", bufs=3) as xp,
              tc.tile_pool(name="probs", bufs=1) as pp,
              tc.tile_pool(name="rot", bufs=2) as rp,
              tc.tile_pool(name="stage", bufs=2) as sg,
              tc.tile_pool(name="ps", bufs=5, space="PSUM") as ps):

            # ---- resident weights ----
            wq_s = wp.tile([128, HC, RQ], BF16)
            wk_s = wp.tile([128, HC, HD], BF16)
            wv_s = wp.tile([128, HC, HD], BF16)
            wo_s = wp.tile([128, QH, H], BF16)
            tm_s = wp.tile([128, 128], BF16)
            ones_s = wp.tile([128, 128], BF16)
            id_s = wp.tile([128, 128], BF16)
            nc.vector.memset(ones_s[:], 1.0)

            # weight chunks on the Sync queue (startup only); x/cc/ss/out
            # on the GpSimd queue whose DMA issue is ~25ns
            _wchunk_loaded = [False] * (HC // 4)
            _tail_loaded = [False]

            def load_wchunk(hg):
                if _wchunk_loaded[hg]:
                    return
                _wchunk_loaded[hg] = True
                if hg == 0:
                    # single-chunk contiguous DMAs so the first matmul's
                    # dependencies land with minimal issue+transfer latency
                    nc.sync.dma_start(out=wq_s[:, 0], in_=wq_e.ap()[0:128])
                    nc.sync.dma_start(out=wk_s[:, 0], in_=wk_e.ap()[0:128])
                    nc.sync.dma_start(out=wv_s[:, 0], in_=wv_e.ap()[0:128])
                    return
                r0, r1 = hg * 512, (hg + 1) * 512
                nc.sync.dma_start(
                    out=wq_s[:, 4 * hg:4 * hg + 4],
                    in_=wq_e.ap()[r0:r1].rearrange("(c p) r -> p c r", c=4))
                nc.sync.dma_start(
                    out=wk_s[:, 4 * hg:4 * hg + 4],
                    in_=wk_e.ap()[r0:r1].rearrange("(c p) r -> p c r", c=4))
                nc.sync.dma_start(
                    out=wv_s[:, 4 * hg:4 * hg + 4],
                    in_=wv_e.ap()[r0:r1].rearrange("(c p) r -> p c r", c=4))

            def load_wchunk0_rest():
                # chunks 1-3, issued after the first x tile's DMA
                nc.sync.dma_start(
                    out=wq_s[:, 1:4],
                    in_=wq_e.ap()[128:512].rearrange("(c p) r -> p c r", c=3))
                nc.sync.dma_start(
                    out=wk_s[:, 1:4],
                    in_=wk_e.ap()[128:512].rearrange("(c p) r -> p c r", c=3))
                nc.sync.dma_start(
                    out=wv_s[:, 1:4],
                    in_=wv_e.ap()[128:512].rearrange("(c p) r -> p c r", c=3))

            def load_tail_weights():
                if _tail_loaded[0]:
                    return
                _tail_loaded[0] = True
                nc.sync.dma_start(out=id_s[:], in_=id_e.ap())
                nc.sync.dma_start(out=tm_s[:], in_=tm_e.ap())
                for rc in range(QH):
                    nc.sync.dma_start(out=wo_s[:, rc],
                                      in_=wo_e.ap()[rc * 128:(rc + 1) * 128])

            _rot_n = [0]

            def rotary(src_ps, s0, ccb, ssb, dst_ap):
                # dst = src*CC + shuffle(src)*SS2, bf16 (DVE 2x mode)
                r = _rot_n[0] = (_rot_n[0] + 1) % 2
                qs = rp.tile([128, SB], BF16, tag=f"qs{r}", bufs=1)
                nc.scalar.activation(qs[:], src_ps[:], COPY)
                qw = rp.tile([128, SB], BF16, tag=f"qw{r}", bufs=1)
                nc.vector.stream_shuffle(qw[:], qs[:], SHUF)
                nc.vector.tensor_mul(qs[:], qs[:], ccb[:, s0:s0 + SB])
                nc.vector.tensor_mul(qw[:], qw[:], ssb[:, s0:s0 + SB])
                nc.vector.tensor_add(dst_ap, qs[:], qw[:])

            pending_wo = []

            og_grp = [None]

            def emit_wo(wb, wib, wat, ho_lo, ho_hi, group=True):
                # 4 ho chunks share one og tile and one grouped output DMA
                wi0 = wib * IB
                for ho in range(ho_lo, min(ho_hi, H // 128)):
                    acc = ps.tile([128, IB], F32, tag="ps", name="acc")
                    for rc in range(QH):
                        nc.tensor.matmul(acc[:],
                                         wo_s[:, rc, ho * 128:(ho + 1) * 128],
                                         wat[:, rc],
                                         start=(rc == 0), stop=(rc == QH - 1))
                    if ho % 4 == 0:
                        og_grp[0] = sg.tile([128, 4, IB], BF16,
                                            tag=f"og{(ho // 4) % 2}",
                                            name="og", bufs=1)
                    og = og_grp[0]
                    if ho % 2 == 0:
                        nc.scalar.activation(og[:, ho % 4], acc[:], COPY)
                    else:
                        nc.vector.tensor_copy(og[:, ho % 4], acc[:])
                    if not group:
                        oeng = nc.scalar if ho % 2 == 0 else nc.sync
                        oeng.dma_start(
                            out=out_e.ap()[wb, ho * 128:(ho + 1) * 128,
                                           wi0:wi0 + IB],
                            in_=og[:, ho % 4])
                    elif ho % 4 == 3:
                        oeng = nc.scalar if (ho // 4) % 2 == 0 else nc.sync
                        oeng.dma_start(
                            out=out_e.ap()[wb, (ho - 3) * 128:(ho + 1) * 128,
                                           wi0:wi0 + IB]
                            .rearrange("(c p) s -> p c s", c=4),
                            in_=og[:])

            for b in range(B):
                qT = st.tile([128, QH, S], BF16, tag="qT")
                kT = st.tile([128, S], BF16, tag="kT")
                vn = st.tile([128, NJT, HD], BF16, tag="vn")
                ccb = st.tile([128, S], BF16, tag="cc")
                ssb = st.tile([128, S], BF16, tag="ss")

                # interleaved: projection block sb feeds attention block
                # ib == sb immediately (causal: ib needs only j <= (sb+1)*SB)
                for sb in range(NSB):
                    s0 = sb * SB
                    # ---- projection for seq block sb ----
                    qa = [ps.tile([128, SB], F32, tag="ps", name=f"qa{rc}")
                          for rc in range(QH)]
                    ka = ps.tile([128, SB], F32, tag="ps")
                    # vta lives in the "sc2" bank: the proj and attention
                    # phases are disjoint, so it never collides with scores
                    vta = ps.tile([128, SB], F32, tag="sc2", bufs=1)
                    for hg in range(HC // 4):
                        first = not _wchunk_loaded[hg] and hg == 0
                        load_wchunk(hg)
                        xt_g = xp.tile([128, 4, SB], BF16, tag=f"xt{hg % 4}",
                                       bufs=1)
                        if b == 0 and sb == 0 and hg <= 1:
                            # split the first x tiles across two queues (two
                            # DMA engines) to halve their transfer latency
                            for c in range(4):
                                eng = nc.sync if c % 2 == 0 else nc.scalar
                                eng.dma_start(
                                    out=xt_g[:, c],
                                    in_=xt_e.ap()[b,
                                                  hg * 512 + c * 128:
                                                  hg * 512 + (c + 1) * 128,
                                                  s0:s0 + SB])
                        else:
                            eng = nc.sync if hg % 2 == 0 else nc.scalar
                            eng.dma_start(
                                out=xt_g[:],
                                in_=xt_e.ap()[b, hg * 512:(hg + 1) * 512,
                                              s0:s0 + SB]
                                .rearrange("(c p) s -> p c s", c=4))
                        if first:
                            load_wchunk0_rest()
                        if sb == 0 and hg == 3:
                            nc.scalar.dma_start(out=ccb[:], in_=cc_e.ap()[b])
                            nc.sync.dma_start(out=ssb[:], in_=ss_e.ap()[b])
                        for c in range(4):
                            hc = hg * 4 + c
                            st_, sp_ = (hc == 0), (hc == HC - 1)
                            for rc in range(QH):
                                nc.tensor.matmul(qa[rc][:],
                                                 wq_s[:, hc, rc * 128:(rc + 1) * 128],
                                                 xt_g[:, c], start=st_, stop=sp_)
                            nc.tensor.matmul(ka[:], wk_s[:, hc], xt_g[:, c],
                                             start=st_, stop=sp_)
                            nc.tensor.matmul(vta[:], wv_s[:, hc], xt_g[:, c],
                                             start=st_, stop=sp_)
                    load_tail_weights()
                    # vt_s copy first on ACT so the V transposes (below,
                    # after the wo slice) never wait on the rotary copies
                    vt_s = rp.tile([128, SB], BF16, tag="vts")
                    nc.scalar.activation(vt_s[:], vta[:], COPY)
                    rotary(ka, s0, ccb, ssb, kT[:, s0:s0 + SB])
                    for rc in range(QH):
                        rotary(qa[rc], s0, ccb, ssb, qT[:, rc, s0:s0 + SB])

                    # previous block's first wo slice: independent PE work
                    # that covers the rotary-chain latency on ACT/DVE; the
                    # rest is interleaved into the attention head loop below
                    prev = pending_wo.pop() if pending_wo else None
                    if prev is not None:
                        emit_wo(*prev, 0, 8)

                    # vT [d, s] -> natural v j-tiles via PE transpose (bf16)
                    for t in range(SB // 128):
                        tp = ps.tile([128, 128], BF16, tag="ps", name="tp",
                                     padded_shape=[128, IB * 2])
                        nc.tensor.transpose(tp[:], vt_s[:, t * 128:(t + 1) * 128],
                                            id_s[:])
                        nc.scalar.activation(vn[:, (SB // 128) * sb + t], tp[:],
                                             COPY)

                    # ---- attention for i block ib == sb ----
                    ib = sb
                    i0 = ib * IB
                    njt = (ib + 1) * (IB // 128)
                    at = ap_.tile([128, QH, IB], BF16, tag="at")
                    sc_n = [0]
                    for h in range(QH):
                        pv = ps.tile([128, IB], F32, tag="ps")
                        sumb = ps.tile([128, IB], F32, tag="ps")
                        pend = []
                        pbs = []
                        sums_ops = []

                        def flush(stop):
                            pjt, pc0, ppb = pend.pop(0)
                            nc.tensor.matmul(pv[:, pc0:], vn[:, pjt],
                                             ppb[:, pc0:],
                                             start=(pjt == 0), stop=stop)
                        for jt in range(njt):
                            kd = jt - ib * 4
                            c0 = kd * 128 if kd > 0 else 0
                            sc = ps.tile([128, IB], F32,
                                         tag=f"sc{sc_n[0] % 3}",
                                         name="sc", bufs=1)
                            sc_n[0] += 1
                            nc.tensor.matmul(sc[:, c0:],
                                             kT[:, jt * 128:(jt + 1) * 128],
                                             qT[:, h, i0 + c0:i0 + IB],
                                             start=True, stop=True)
                            if kd >= 0:
                                # causal mask: accumulate a -60 upper triangle
                                # into the boundary 128 columns on the PE, so
                                # exp zeroes it — no DVE op, no mask->PV dep
                                nc.tensor.matmul(sc[:, c0:c0 + 128], id_s[:],
                                                 tm_s[:], start=False,
                                                 stop=True,
                                                 skip_group_check=True)
                            pb = pp.tile([128, IB], BF16, tag=f"pb{jt}",
                                         bufs=1)
                            nc.scalar.activation(pb[:, c0:], sc[:, c0:], EXP)
                            pend.append((jt, c0, pb))
                            pbs.append(pb)
                            sums_ops.append((c0, pb))
                            if len(pend) > 2:
                                flush(False)
                        while pend:
                            flush(stop=(len(pend) == 1))
                        # softmax denominators: all-ones stationary sums pb
                        # over j, broadcast to all 128 partitions.  Pure PE
                        # work, scheduled after PV so it fills exp bubbles.
                        for k, (c0, src) in enumerate(sums_ops):
                            nc.tensor.matmul(sumb[:, c0:], ones_s[:],
                                             src[:, c0:],
                                             start=(k == 0),
                                             stop=(k == len(sums_ops) - 1))
                        rb = sg.tile([128, IB], F32, tag=f"rb{h % 2}", bufs=1)
                        nc.vector.reciprocal_approx_fast(rb[:], sumb[:])
                        nc.vector.tensor_mul(at[:, h], pv[:], rb[:])
                        # next slice of the previous block's wo: PE slack
                        # work covering this head's recip/normalize chain
                        if prev is not None:
                            emit_wo(*prev, 8 * (h + 1), 8 * (h + 2))
                    pending_wo.append((b, ib, at))

            emit_wo(*pending_wo.pop(), 0, H // 128, group=False)

    nc.compile()
    return nc


def _prep(x, freqs_cos, freqs_sin, wq, wk, wv, wo):
    """Shard + pre-transpose inputs for the 8 cores."""
    # rotary pair permutation: within each 32-partition quadrant, x0 of
    # pairs [16q,16q+16) sits in local slots 0..15 and x1 in 16..31.
    perm = np.zeros(HD, dtype=np.int64)
    pair = np.zeros(128, dtype=np.int64)
    sign = np.zeros(128, dtype=np.float32)
    for q in range(4):
        for t in range(16):
            perm[32 * q + t] = 2 * (16 * q + t)
            perm[32 * q + 16 + t] = 2 * (16 * q + t) + 1
            pair[32 * q + t] = 16 * q + t
            pair[32 * q + 16 + t] = 16 * q + t
            sign[32 * q + t] = -1.0
            sign[32 * q + 16 + t] = 1.0

    xt = np.ascontiguousarray(x.transpose(0, 2, 1)).astype(BF16NP)  # [B,H,S]
    cc = np.ascontiguousarray(freqs_cos[:, :, pair].transpose(0, 2, 1)).astype(BF16NP)
    ss2 = np.ascontiguousarray((freqs_sin[:, :, pair] * sign).transpose(0, 2, 1)).astype(BF16NP)

    # additive causal mask for the 128-wide boundary sub-tile of a diagonal
    # j tile: 0 where j <= i, -60 above (exp(-60+s) == 0 in bf16/f32)
    jj = np.arange(128)[:, None]
    ii = np.arange(128)[None, :]
    trimask = np.where(jj <= ii, 0.0, -60.0).astype(BF16NP)

    scale = np.float32(1.0 / np.sqrt(HD))
    in_maps = []
    for c in range(NCORES):
        wq_c = (wq[c * RQ:(c + 1) * RQ] * scale).reshape(QH, HD, H)[:, perm, :]
        wqt = np.ascontiguousarray(wq_c.reshape(RQ, H).T).astype(BF16NP)
        wk_c = wk[c * HD:(c + 1) * HD][perm, :]
        wkt = np.ascontiguousarray(wk_c.T).astype(BF16NP)
        wvt = np.ascontiguousarray(wv[c * HD:(c + 1) * HD].T).astype(BF16NP)
        wot = np.ascontiguousarray(wo[:, c * RQ:(c + 1) * RQ].T).astype(BF16NP)
        in_maps.append({
            "xt": xt, "cc": cc, "ss2": ss2,
            "wqt": wqt, "wkt": wkt, "wvt": wvt, "wot": wot,
            "trimask": trimask, "ident": np.eye(128, dtype=BF16NP),
        })
    return in_maps


def kernel(x, freqs_cos, freqs_sin, wq, wk, wv, wo):
    global _CACHED, LAST_EXEC_NS
    x = np.asarray(x, dtype=np.float32)
    freqs_cos = np.asarray(freqs_cos, dtype=np.float32)
    freqs_sin = np.asarray(freqs_sin, dtype=np.float32)
    wq = np.asarray(wq, dtype=np.float32)
    wk = np.asarray(wk, dtype=np.float32)
    wv = np.asarray(wv, dtype=np.float32)
    wo = np.asarray(wo, dtype=np.float32)

    if _CACHED is None:
        _CACHED = _build()
    nc = _CACHED

    in_maps = _prep(x, freqs_cos, freqs_sin, wq, wk, wv, wo)
    res = run_bass_kernel_spmd(nc, in_maps, core_ids=list(range(NCORES)))
    LAST_EXEC_NS = res.exec_time_ns

    # unshard: sum the 8 partial wo products, then [B,H,S] -> [B,S,H]
    acc = res.results[0]["out"].astype(np.float64)
    for c in range(1, NCORES):
        acc += res.results[c]["out"]
    return np.ascontiguousarray(acc.transpose(0, 2, 1)).astype(np.float32)
